# revision 12
# baseline (speedup 1.0000x reference)
"""GCN forward entirely on 8 TRN2 NeuronCores.

Nodes are row-sharded (12500/core). Per layer, per core:
  - dma_gather pulls bf16 messages hs[src] from a replicated hs table in
    DRAM (4 int16 index windows of 32768 rows, 8192-token chunks).
  - segment-sum runs on the tensor engine: for each 128-dst window a PSUM
    tile [feat, seg] accumulates G_col^T @ S_col, where the selector
    S[p, s] = (seg[p] == s) * norm_d[dst[p]] is built by one DVE
    tensor_scalar (is_equal x mult) per 128-token column. No scatter DMA.
  - dense: z^T = W^T aggT (+bias, relu) on PE/ACT, residual add on DVE.
  - hs_own = h * norm_s (PE transpose + ACT scale, bf16) -> AllGather into
    every core's hs table for the next layer.
Host does index packing / norm precompute / final transpose only.
"""
import sys
import time
import types
import numpy as np
import ml_dtypes

import concourse.bass as bass
import concourse.bacc as bacc
import concourse.tile as tile
from concourse import mybir
from concourse.bass_utils import run_bass_kernel_spmd

N = 100000
E = 1600000
D = 128
L = 4
NC = 8
R = N // NC              # 12500 nodes per core
NT = 98                  # 128-dst windows per core (97*128+84)
PN = NT * 128            # padded rows per core in the hs table (12544)
NP = NC * PN             # padded hs table rows (100352)
WINB = 32768             # gather index window (int16 range)
NB = 4                   # src buckets (ceil(NP / WINB))
CHG = 8192               # gather chunk tokens
TCH = 25                 # dense matmul chunks of 500 cols

bf16 = mybir.dt.bfloat16
f32 = mybir.dt.float32
i16 = mybir.dt.int16
AF = mybir.ActivationFunctionType
ALU = mybir.AluOpType
bfd = ml_dtypes.bfloat16

_cache = {}
LAST_RESULT = None


def _build(K_WB, NCHB):
    BPAD = NCHB * CHG            # tokens per bucket stream (padded)
    TOK = NB * BPAD
    TOTCOL = TOK // 128
    BCOL = BPAD // 128           # columns per bucket
    nc = bacc.Bacc("TRN2", target_bir_lowering=False, debug=False, num_devices=NC)

    xT_d = nc.dram_tensor("xT", [D, R], f32, kind="ExternalInput")
    we_d = nc.dram_tensor("we", [D, D], f32, kind="ExternalInput")       # W_embed[k,d]
    ws_d = nc.dram_tensor("ws", [D, L, D], bf16, kind="ExternalInput")   # [k,l,d]
    bias_d = nc.dram_tensor("bias", [D, L + 1], f32, kind="ExternalInput")
    ns_d = nc.dram_tensor("ns", [128, NT], f32, kind="ExternalInput")    # norm_s own
    iota_d = nc.dram_tensor("iota", [128, 128], bf16, kind="ExternalInput")
    ident_d = nc.dram_tensor("ident", [128, 128], f32, kind="ExternalInput")
    gidx_d = nc.dram_tensor("gidx", [128, TOK // 16], i16, kind="ExternalInput")
    segf_d = nc.dram_tensor("segf", [128, TOTCOL], f32, kind="ExternalInput")
    ndtok_d = nc.dram_tensor("ndtok", [128, TOTCOL], f32, kind="ExternalInput")
    outT_d = nc.dram_tensor("outT", [D, R], f32, kind="ExternalOutput")

    hs_d = nc.dram_tensor("hs", [NP, D], bf16, kind="Internal", addr_space="Shared")
    agin_d = nc.dram_tensor("agin", [PN, D], bf16, kind="Internal")

    with tile.TileContext(nc, num_cores=NC) as tc:
        with (
            tc.tile_pool(name="persist", bufs=1) as pp,
            tc.tile_pool(name="work", bufs=3) as wp,
            tc.tile_pool(name="sel", bufs=4) as sp_,
            tc.tile_pool(name="msgp", bufs=3) as mp,
            tc.tile_pool(name="pw", bufs=3, space="PSUM") as pwp,
            tc.tile_pool(name="pz", bufs=2, space="PSUM") as pzp,
        ):
            hT = pp.tile([D, R], f32)
            aggT = pp.tile([D, NT * 128], bf16)
            gidx = pp.tile([128, TOK // 16], i16)
            segf = pp.tile([128, TOTCOL], f32)
            ndtok = pp.tile([128, TOTCOL], f32)
            ws = pp.tile([D, L, D], bf16)
            we = pp.tile([D, D], f32)
            biases = pp.tile([D, L + 1], f32)
            ns = pp.tile([128, NT], f32)
            iota = pp.tile([128, 128], bf16)
            ident = pp.tile([128, 128], f32)

            nc.sync.dma_start(gidx[:], gidx_d[:])
            nc.sync.dma_start(segf[:], segf_d[:])
            nc.sync.dma_start(ndtok[:], ndtok_d[:])
            nc.sync.dma_start(ws[:], ws_d[:])
            nc.sync.dma_start(we[:], we_d[:])
            nc.sync.dma_start(biases[:], bias_d[:])
            nc.sync.dma_start(ns[:], ns_d[:])
            nc.sync.dma_start(iota[:], iota_d[:])
            nc.sync.dma_start(ident[:], ident_d[:])

            # ---- embed: hT = We^T @ xT + b_embed ----
            for t in range(TCH):
                xc = wp.tile([D, 500], f32, tag="xc")
                nc.sync.dma_start(xc[:], xT_d[:, t * 500:(t + 1) * 500])
                pz = pzp.tile([128, 500], f32, tag="pz")
                nc.tensor.matmul(pz[:], we[:], xc[:], start=True, stop=True)
                nc.scalar.activation(hT[:, t * 500:(t + 1) * 500], pz[:],
                                     AF.Identity, bias=biases[:, 0:1])

            def emit_hs():
                """agin = (h * norm_s) as bf16 [node, feat]; AllGather -> hs."""
                for t8 in range((NT + 7) // 8):
                    nt8 = min(8, NT - t8 * 8)
                    st = wp.tile([128, 8, 128], bf16, tag="hst")
                    for j in range(nt8):
                        t = t8 * 8 + j
                        cw = min(128, R - t * 128)
                        pt = pwp.tile([128, 128], f32, tag="pt")
                        nc.tensor.transpose(pt[:cw, :], hT[:, t * 128:t * 128 + cw],
                                            ident[:])
                        nc.scalar.activation(st[:cw, j, :], pt[:cw, :], AF.Copy,
                                             scale=ns[:cw, t:t + 1])
                    dst = agin_d[:].rearrange("(a p) d -> p a d", p=128)
                    nc.sync.dma_start(dst[:, t8 * 8:t8 * 8 + nt8, :],
                                      st[:, :nt8, :])
                nc.gpsimd.collective_compute(
                    "AllGather", ALU.bypass,
                    replica_groups=[list(range(NC))],
                    ins=[agin_d[:]], outs=[hs_d[:]],
                )

            emit_hs()

            for l in range(L):
                # ---- aggregation: aggT[feat, node] via selector matmuls ----
                msg_tiles = {}

                def get_chunk(g):
                    if g not in msg_tiles:
                        b = g // NCHB
                        lo, hi = b * WINB, min(NP, (b + 1) * WINB)
                        m = mp.tile([128, 64, D], bf16, tag="msg")
                        sl = slice(g * CHG // 16, (g + 1) * CHG // 16)
                        nc.gpsimd.dma_gather(m[:], hs_d[lo:hi, :], gidx[:, sl],
                                             CHG, CHG, D, single_packet=False)
                        msg_tiles[g] = m
                    return msg_tiles[g]

                for b in range(NB):
                    for w in range(NT):
                        pw = pwp.tile([128, 128], f32, tag="pw")
                        for k in range(K_WB):
                            col = b * BCOL + w * K_WB + k
                            m = get_chunk(col // 64)
                            S = sp_.tile([128, 128], bf16, tag="S")
                            nc.vector.tensor_scalar(
                                S[:], iota[:], segf[:, col:col + 1],
                                ndtok[:, col:col + 1],
                                op0=ALU.is_equal, op1=ALU.mult)
                            nc.tensor.matmul(pw[:], m[:, col % 64, :], S[:],
                                             start=(k == 0), stop=(k == K_WB - 1))
                        if b == 0:
                            nc.scalar.activation(aggT[:, w * 128:(w + 1) * 128],
                                                 pw[:], AF.Copy)
                        else:
                            nc.vector.tensor_tensor(
                                aggT[:, w * 128:(w + 1) * 128],
                                aggT[:, w * 128:(w + 1) * 128], pw[:],
                                op=ALU.add)

                # ---- dense: h += relu(W_l^T aggT + b_l) ----
                for t in range(TCH):
                    pz = pzp.tile([128, 500], f32, tag="pz")
                    nc.tensor.matmul(pz[:], ws[:, l, :],
                                     aggT[:, t * 500:(t + 1) * 500],
                                     start=True, stop=True)
                    rl = wp.tile([128, 500], f32, tag="rl")
                    nc.scalar.activation(rl[:], pz[:], AF.Relu,
                                         bias=biases[:, l + 1:l + 2])
                    nc.vector.tensor_tensor(hT[:, t * 500:(t + 1) * 500],
                                            hT[:, t * 500:(t + 1) * 500], rl[:],
                                            op=ALU.add)
                if l < L - 1:
                    emit_hs()

            nc.sync.dma_start(outT_d[:], hT[:])

    t1 = time.time()
    nc.compile()
    print(f"[kernel] bacc compile: {time.time() - t1:.1f}s", file=sys.stderr)
    return nc


def _install_profile_hook():
    if "antenv.axon_hooks" in sys.modules:
        return
    try:
        import antenv
        import trn_agent_boot.trn_boot as tb
        hook = tb._ntff_profile_via_ctypes("/opt/axon/libaxon_pjrt.so")
        mod = types.ModuleType("antenv.axon_hooks")
        mod.get_axon_ntff_profile_hook = lambda: hook
        sys.modules["antenv.axon_hooks"] = mod
        antenv.axon_hooks = mod
    except Exception as e:
        print(f"[kernel] profile hook unavailable: {e}", file=sys.stderr)


def _prep_core(c, src, dst, norm_s, norm_d):
    m = (dst >= c * R) & (dst < (c + 1) * R)
    es = src[m]
    ed = dst[m] - c * R
    es_pad = (es // R) * PN + (es % R)   # index into padded hs table
    b = es_pad >> 15
    w = ed >> 7
    seg = ed & 127
    gkey = b * NT + w
    order = np.argsort(gkey, kind="stable")
    counts = np.bincount(gkey, minlength=NB * NT)
    return es_pad[order], ed[order], b[order], seg[order], gkey[order], counts


def kernel(h, src, dst, W_embed, b_embed, Ws, bs):
    h = np.asarray(h, np.float32)
    src = np.asarray(src).astype(np.int64)
    dst = np.asarray(dst).astype(np.int64)
    W_embed = np.asarray(W_embed, np.float32)
    b_embed = np.asarray(b_embed, np.float32)
    Ws = np.asarray(Ws, np.float32)
    bs = np.asarray(bs, np.float32)

    deg_out = np.bincount(src, minlength=N).astype(np.float32)
    deg_in = np.bincount(dst, minlength=N).astype(np.float32)
    norm_s = 1.0 / np.sqrt(np.maximum(deg_out, 1.0))
    norm_d = 1.0 / np.sqrt(np.maximum(deg_in, 1.0))

    per_core = [_prep_core(c, src, dst, norm_s, norm_d) for c in range(NC)]
    K_WB = int(np.ceil(max(pc[5].max() for pc in per_core) / 128))
    SLOT = K_WB * 128
    BSTREAM = NT * SLOT
    NCHB = int(np.ceil(BSTREAM / CHG))
    BPAD = NCHB * CHG
    TOK = NB * BPAD
    TOTCOL = TOK // 128

    key = (K_WB, NCHB)
    if key not in _cache:
        t0 = time.time()
        _cache[key] = _build(K_WB, NCHB)
        print(f"[kernel] build total: {time.time() - t0:.1f}s", file=sys.stderr)
    nc = _cache[key]

    iota_np = np.tile(np.arange(128, dtype=bfd)[None, :], (128, 1))
    ident_np = np.eye(128, dtype=np.float32)
    ws_packed = np.ascontiguousarray(Ws.transpose(1, 0, 2)).astype(bfd)
    bias_packed = np.concatenate([b_embed[:, None], bs.T], axis=1).astype(np.float32)

    in_maps = []
    for c in range(NC):
        es, ed, b, seg, gkey, counts = per_core[c]
        starts = np.zeros(NB * NT, np.int64)
        starts[1:] = np.cumsum(counts)[:-1]
        rank = np.arange(len(es)) - starts[gkey]
        pos_b = gkey // NT
        pos = pos_b * BPAD + (gkey % NT) * SLOT + rank

        gfull = np.zeros(TOK, np.int16)
        sfull = np.full(TOK, -1.0, np.float32)
        ndfull = np.zeros(TOK, np.float32)
        gfull[pos] = (es - (b << 15)).astype(np.int16)  # es is already padded-space
        sfull[pos] = seg
        nd_local = norm_d[c * R:(c + 1) * R]
        ndfull[pos] = nd_local[ed]

        gidx = np.tile(np.ascontiguousarray(gfull.reshape(-1, 16).T), (8, 1))
        segf = np.ascontiguousarray(sfull.reshape(TOTCOL, 128).T).astype(np.float32)
        ndtok = np.ascontiguousarray(ndfull.reshape(TOTCOL, 128).T).astype(np.float32)

        ns_pad = np.pad(norm_s[c * R:(c + 1) * R], (0, NT * 128 - R))
        in_maps.append({
            "xT": np.ascontiguousarray(h[c * R:(c + 1) * R, :].T),
            "we": np.ascontiguousarray(W_embed),
            "ws": ws_packed,
            "bias": bias_packed,
            "ns": np.ascontiguousarray(ns_pad.reshape(NT, 128).T.astype(np.float32)),
            "iota": iota_np,
            "ident": ident_np,
            "gidx": gidx,
            "segf": segf,
            "ndtok": ndtok,
        })

    global LAST_RESULT
    _install_profile_hook()
    t0 = time.time()
    res = run_bass_kernel_spmd(nc, in_maps, list(range(NC)))
    print(f"[kernel] spmd run: {time.time() - t0:.1f}s", file=sys.stderr)
    LAST_RESULT = res
    out = np.concatenate([res.results[c]["outT"].T for c in range(NC)], axis=0)
    return np.ascontiguousarray(out.astype(np.float32))


# revision 16
# speedup vs baseline: 1.2811x; 1.2811x over previous
"""GCN forward entirely on 8 TRN2 NeuronCores.

Nodes are row-sharded (12500/core). Per layer, per core:
  - dma_gather pulls bf16 messages hs[src] from a replicated hs table in
    DRAM (4 int16 index windows of 32768 rows, 8192-token chunks).
  - segment-sum runs on the tensor engine: for each 128-dst window a PSUM
    tile [feat, seg] accumulates G_col^T @ S_col, where the selector
    S[p, s] = (seg[p] == s) * norm_d[dst[p]] is built by one DVE
    tensor_scalar (is_equal x mult) per 128-token column. No scatter DMA.
  - dense: z^T = W^T aggT (+bias, relu) on PE/ACT, residual add on DVE.
  - hs_own = h * norm_s (PE transpose + ACT scale, bf16) -> AllGather into
    every core's hs table for the next layer.
Host does index packing / norm precompute / final transpose only.
"""
import sys
import time
import types
import numpy as np
import ml_dtypes

import concourse.bass as bass
import concourse.bacc as bacc
import concourse.tile as tile
from concourse import mybir
from concourse.bass_utils import run_bass_kernel_spmd

N = 100000
E = 1600000
D = 128
L = 4
NC = 8
R = N // NC              # 12500 nodes per core
NT = 98                  # 128-dst windows per core (97*128+84)
PN = NT * 128            # padded rows per core in the hs table (12544)
NP = NC * PN             # padded hs table rows (100352)
WINB = 32768             # gather index window (int16 range)
NB = 4                   # src buckets (ceil(NP / WINB))
CHG = 8192               # gather chunk tokens
TCH = 25                 # dense matmul chunks of 500 cols

bf16 = mybir.dt.bfloat16
f32 = mybir.dt.float32
i16 = mybir.dt.int16
AF = mybir.ActivationFunctionType
ALU = mybir.AluOpType
bfd = ml_dtypes.bfloat16

_cache = {}
LAST_RESULT = None


def _build(K_WB, NCHB):
    BPAD = NCHB * CHG            # tokens per bucket stream (padded)
    TOK = NB * BPAD
    TOTCOL = TOK // 128
    BCOL = BPAD // 128           # columns per bucket
    nc = bacc.Bacc("TRN2", target_bir_lowering=False, debug=False, num_devices=NC)

    xT_d = nc.dram_tensor("xT", [D, R], f32, kind="ExternalInput")
    we_d = nc.dram_tensor("we", [D, D], f32, kind="ExternalInput")       # W_embed[k,d]
    ws_d = nc.dram_tensor("ws", [D, L, D], bf16, kind="ExternalInput")   # [k,l,d]
    bias_d = nc.dram_tensor("bias", [D, L + 1], f32, kind="ExternalInput")
    ns_d = nc.dram_tensor("ns", [128, NT], f32, kind="ExternalInput")    # norm_s own
    iota_d = nc.dram_tensor("iota", [128, 128], f32, kind="ExternalInput")
    ident_d = nc.dram_tensor("ident", [128, 128], f32, kind="ExternalInput")
    gidx_d = nc.dram_tensor("gidx", [128, TOK // 16], i16, kind="ExternalInput")
    segf_d = nc.dram_tensor("segf", [128, TOTCOL], f32, kind="ExternalInput")
    ndtok_d = nc.dram_tensor("ndtok", [128, TOTCOL], f32, kind="ExternalInput")
    outT_d = nc.dram_tensor("outT", [D, R], f32, kind="ExternalOutput")

    hs_d = nc.dram_tensor("hs", [NP, D], bf16, kind="Internal", addr_space="Shared")
    agin_d = nc.dram_tensor("agin", [PN, D], bf16, kind="Internal")

    with tile.TileContext(nc, num_cores=NC) as tc:
        with (
            tc.tile_pool(name="persist", bufs=1) as pp,
            tc.tile_pool(name="work", bufs=3) as wp,
            tc.tile_pool(name="sel", bufs=2) as sp_,
            tc.tile_pool(name="msgp", bufs=2) as mp,
            tc.tile_pool(name="pw", bufs=3, space="PSUM") as pwp,
            tc.tile_pool(name="pz", bufs=2, space="PSUM") as pzp,
        ):
            hT = pp.tile([D, R], f32)
            aggT = pp.tile([D, NT * 128], bf16)
            gidx = pp.tile([128, TOK // 16], i16)
            segf = pp.tile([128, TOTCOL], f32)
            ndtok = pp.tile([128, TOTCOL], f32)
            ws = pp.tile([D, L, D], bf16)
            we = pp.tile([D, D], f32)
            biases = pp.tile([D, L + 1], f32)
            ns = pp.tile([128, NT], f32)
            iota = pp.tile([128, 128], f32)
            ident = pp.tile([128, 128], f32)

            nc.sync.dma_start(gidx[:], gidx_d[:])
            nc.sync.dma_start(segf[:], segf_d[:])
            nc.sync.dma_start(ndtok[:], ndtok_d[:])
            nc.sync.dma_start(ws[:], ws_d[:])
            nc.sync.dma_start(we[:], we_d[:])
            nc.sync.dma_start(biases[:], bias_d[:])
            nc.sync.dma_start(ns[:], ns_d[:])
            nc.sync.dma_start(iota[:], iota_d[:])
            nc.sync.dma_start(ident[:], ident_d[:])

            # ---- embed: hT = We^T @ xT + b_embed ----
            for t in range(TCH):
                xc = wp.tile([D, 500], f32, tag="xc")
                nc.sync.dma_start(xc[:], xT_d[:, t * 500:(t + 1) * 500])
                pz = pzp.tile([128, 500], f32, tag="pz")
                nc.tensor.matmul(pz[:], we[:], xc[:], start=True, stop=True)
                nc.scalar.activation(hT[:, t * 500:(t + 1) * 500], pz[:],
                                     AF.Identity, bias=biases[:, 0:1])

            def emit_hs():
                """agin = (h * norm_s) as bf16 [node, feat]; AllGather -> hs."""
                for t8 in range((NT + 7) // 8):
                    nt8 = min(8, NT - t8 * 8)
                    st = wp.tile([128, 8, 128], bf16, tag="hst")
                    for j in range(nt8):
                        t = t8 * 8 + j
                        cw = min(128, R - t * 128)
                        pt = pwp.tile([128, 128], f32, tag="pt")
                        nc.tensor.transpose(pt[:cw, :], hT[:, t * 128:t * 128 + cw],
                                            ident[:])
                        nc.scalar.activation(st[:cw, j, :], pt[:cw, :], AF.Copy,
                                             scale=ns[:cw, t:t + 1])
                    dst = agin_d[:].rearrange("(a p) d -> p a d", p=128)
                    nc.sync.dma_start(dst[:, t8 * 8:t8 * 8 + nt8, :],
                                      st[:, :nt8, :])
                nc.gpsimd.collective_compute(
                    "AllGather", ALU.bypass,
                    replica_groups=[list(range(NC))],
                    ins=[agin_d[:]], outs=[hs_d[:]],
                )

            emit_hs()

            for l in range(L):
                # ---- aggregation: aggT[feat, node] via selector matmuls ----
                msg_tiles = {}

                def get_chunk(g):
                    if g not in msg_tiles:
                        b = g // NCHB
                        lo, hi = b * WINB, min(NP, (b + 1) * WINB)
                        # last chunk of a bucket stream only carries the
                        # real columns (NT*K_WB*128 tokens per bucket)
                        n = min(CHG, NT * K_WB * 128 - (g % NCHB) * CHG)
                        m = mp.tile([128, 64, D], bf16, tag="msg")
                        sl = slice(g * CHG // 16, (g * CHG + n) // 16)
                        nc.gpsimd.dma_gather(m[:, :n // 128, :], hs_d[lo:hi, :],
                                             gidx[:, sl], n, n, D,
                                             single_packet=False)
                        msg_tiles[g] = m
                    return msg_tiles[g]

                for b in range(NB):
                    for w in range(NT):
                        pw = pwp.tile([128, 128], f32, tag="pw")
                        c0 = b * BCOL + w * K_WB
                        T1 = sp_.tile([128, K_WB, 128], f32, tag="T1")
                        S = sp_.tile([128, K_WB, 128], bf16, tag="S")
                        nc.vector.tensor_tensor(
                            T1[:],
                            segf[:, c0:c0 + K_WB].unsqueeze(2)
                                .broadcast_to([128, K_WB, 128]),
                            iota[:].unsqueeze(1).broadcast_to([128, K_WB, 128]),
                            op=ALU.is_equal)
                        nc.vector.tensor_tensor(
                            S[:], T1[:],
                            ndtok[:, c0:c0 + K_WB].unsqueeze(2)
                                .broadcast_to([128, K_WB, 128]),
                            op=ALU.mult)
                        for k in range(K_WB):
                            col = c0 + k
                            m = get_chunk(col // 64)
                            nc.tensor.matmul(pw[:], m[:, col % 64, :], S[:, k, :],
                                             start=(k == 0), stop=(k == K_WB - 1))
                        if b == 0:
                            nc.scalar.activation(aggT[:, w * 128:(w + 1) * 128],
                                                 pw[:], AF.Copy)
                        else:
                            nc.vector.tensor_tensor(
                                aggT[:, w * 128:(w + 1) * 128],
                                aggT[:, w * 128:(w + 1) * 128], pw[:],
                                op=ALU.add)

                # ---- dense: h += relu(W_l^T aggT + b_l) ----
                for t in range(TCH):
                    pz = pzp.tile([128, 500], f32, tag="pz")
                    nc.tensor.matmul(pz[:], ws[:, l, :],
                                     aggT[:, t * 500:(t + 1) * 500],
                                     start=True, stop=True)
                    rl = wp.tile([128, 500], f32, tag="rl")
                    nc.scalar.activation(rl[:], pz[:], AF.Relu,
                                         bias=biases[:, l + 1:l + 2])
                    nc.vector.tensor_tensor(hT[:, t * 500:(t + 1) * 500],
                                            hT[:, t * 500:(t + 1) * 500], rl[:],
                                            op=ALU.add)
                if l < L - 1:
                    emit_hs()

            nc.sync.dma_start(outT_d[:], hT[:])

    t1 = time.time()
    nc.compile()
    print(f"[kernel] bacc compile: {time.time() - t1:.1f}s", file=sys.stderr)
    return nc


def _install_profile_hook():
    if "antenv.axon_hooks" in sys.modules:
        return
    try:
        import antenv
        import trn_agent_boot.trn_boot as tb
        hook = tb._ntff_profile_via_ctypes("/opt/axon/libaxon_pjrt.so")
        mod = types.ModuleType("antenv.axon_hooks")
        mod.get_axon_ntff_profile_hook = lambda: hook
        sys.modules["antenv.axon_hooks"] = mod
        antenv.axon_hooks = mod
    except Exception as e:
        print(f"[kernel] profile hook unavailable: {e}", file=sys.stderr)


def _prep_core(c, src, dst, norm_s, norm_d):
    m = (dst >= c * R) & (dst < (c + 1) * R)
    es = src[m]
    ed = dst[m] - c * R
    es_pad = (es // R) * PN + (es % R)   # index into padded hs table
    b = es_pad >> 15
    w = ed >> 7
    seg = ed & 127
    gkey = b * NT + w
    order = np.argsort(gkey, kind="stable")
    counts = np.bincount(gkey, minlength=NB * NT)
    return es_pad[order], ed[order], b[order], seg[order], gkey[order], counts


def kernel(h, src, dst, W_embed, b_embed, Ws, bs):
    h = np.asarray(h, np.float32)
    src = np.asarray(src).astype(np.int64)
    dst = np.asarray(dst).astype(np.int64)
    W_embed = np.asarray(W_embed, np.float32)
    b_embed = np.asarray(b_embed, np.float32)
    Ws = np.asarray(Ws, np.float32)
    bs = np.asarray(bs, np.float32)

    deg_out = np.bincount(src, minlength=N).astype(np.float32)
    deg_in = np.bincount(dst, minlength=N).astype(np.float32)
    norm_s = 1.0 / np.sqrt(np.maximum(deg_out, 1.0))
    norm_d = 1.0 / np.sqrt(np.maximum(deg_in, 1.0))

    per_core = [_prep_core(c, src, dst, norm_s, norm_d) for c in range(NC)]
    K_WB = int(np.ceil(max(pc[5].max() for pc in per_core) / 128))
    SLOT = K_WB * 128
    BSTREAM = NT * SLOT
    NCHB = int(np.ceil(BSTREAM / CHG))
    BPAD = NCHB * CHG
    TOK = NB * BPAD
    TOTCOL = TOK // 128

    key = (K_WB, NCHB)
    if key not in _cache:
        t0 = time.time()
        _cache[key] = _build(K_WB, NCHB)
        print(f"[kernel] build total: {time.time() - t0:.1f}s", file=sys.stderr)
    nc = _cache[key]

    iota_np = np.tile(np.arange(128, dtype=np.float32)[None, :], (128, 1))
    ident_np = np.eye(128, dtype=np.float32)
    ws_packed = np.ascontiguousarray(Ws.transpose(1, 0, 2)).astype(bfd)
    bias_packed = np.concatenate([b_embed[:, None], bs.T], axis=1).astype(np.float32)

    in_maps = []
    for c in range(NC):
        es, ed, b, seg, gkey, counts = per_core[c]
        starts = np.zeros(NB * NT, np.int64)
        starts[1:] = np.cumsum(counts)[:-1]
        rank = np.arange(len(es)) - starts[gkey]
        pos_b = gkey // NT
        pos = pos_b * BPAD + (gkey % NT) * SLOT + rank

        gfull = np.zeros(TOK, np.int16)
        sfull = np.full(TOK, -1.0, np.float32)
        ndfull = np.zeros(TOK, np.float32)
        gfull[pos] = (es - (b << 15)).astype(np.int16)  # es is already padded-space
        sfull[pos] = seg
        nd_local = norm_d[c * R:(c + 1) * R]
        ndfull[pos] = nd_local[ed]

        gidx = np.tile(np.ascontiguousarray(gfull.reshape(-1, 16).T), (8, 1))
        segf = np.ascontiguousarray(sfull.reshape(TOTCOL, 128).T).astype(np.float32)
        ndtok = np.ascontiguousarray(ndfull.reshape(TOTCOL, 128).T).astype(np.float32)

        ns_pad = np.pad(norm_s[c * R:(c + 1) * R], (0, NT * 128 - R))
        in_maps.append({
            "xT": np.ascontiguousarray(h[c * R:(c + 1) * R, :].T),
            "we": np.ascontiguousarray(W_embed),
            "ws": ws_packed,
            "bias": bias_packed,
            "ns": np.ascontiguousarray(ns_pad.reshape(NT, 128).T.astype(np.float32)),
            "iota": iota_np,
            "ident": ident_np,
            "gidx": gidx,
            "segf": segf,
            "ndtok": ndtok,
        })

    global LAST_RESULT
    _install_profile_hook()
    t0 = time.time()
    res = run_bass_kernel_spmd(nc, in_maps, list(range(NC)))
    print(f"[kernel] spmd run: {time.time() - t0:.1f}s", file=sys.stderr)
    LAST_RESULT = res
    out = np.concatenate([res.results[c]["outT"].T for c in range(NC)], axis=0)
    return np.ascontiguousarray(out.astype(np.float32))
